# Initial kernel scaffold
#
"""Trainium2 Bass kernel for a collision-grid social-LSTM model.

Math per frame t (N=512 agents, V=64 vehicles):
  social   = max_j grids_TTC[t, :, j, :]          # [N, 24]
  social_v = max_j grids_TTC_veh[t, :, j, :]      # [N, 24]
  e_in = relu(nodes @ W_in + b_in)                # nodes = input_data[:, [0,1,5..8]]
  e_t  = relu(social @ W_t + b_t)
  e_tv = relu(social_v @ W_tv + b_tv)
  gates = [e_in e_t e_tv] @ W_ih + b_ih + h @ W_hh + b_hh
  LSTM cell (i,f,g,o) -> h, c;  out = h @ W_out + b_out

Sharding: agent dim N split across 8 NeuronCores (64 rows each); weights
replicated; T-scan sequential per core; no collectives needed.

Per-core grid layout: frame slab [64i, 512j, 24s] is reshaped on host to
[128, 6144] with partition p = (j_half*64 + i) so the DMA is contiguous and
the j-reduction uses all 128 partitions.  A PE transpose + elementwise max
merges the two j-halves and yields social^T [24, 64] directly, which is the
operand orientation every downstream matmul wants.
"""

import numpy as np

import concourse.tile as tile
from concourse import bacc, mybir

T, N, V = 19, 512, 64
F, E, R, O = 9, 128, 256, 5
S = 24
NCORES = 8
NL = N // NCORES          # 64 agent rows per core
ROWS = T * NL             # 1216 (t-major row index = t*NL + i)
PFREE = (N // 2) * S      # 6144 free elems per partition (ped)
VFREE = (V // 2) * S      # 768 (veh)

DT = mybir.dt.float32
GRID_DT = mybir.dt.float32   # dtype grids are staged in device DRAM
GRID_NP = np.float32

_NC_CACHE = {}


def build_nc(repeat=1):
    """Build + compile the per-core Bass module (identical on all cores)."""
    if repeat in _NC_CACHE:
        return _NC_CACHE[repeat]

    nc = bacc.Bacc("TRN2", target_bir_lowering=False, debug=False,
                   num_devices=NCORES)
    dt = DT
    AF = mybir.ActivationFunctionType
    ALU = mybir.AluOpType
    AX = mybir.AxisListType

    # ---- DRAM I/O ----
    g_ped = nc.dram_tensor("g_ped", [T, 128, PFREE], GRID_DT, kind="ExternalInput")
    g_veh = nc.dram_tensor("g_veh", [T, 128, VFREE], GRID_DT, kind="ExternalInput")
    nodes_T = nc.dram_tensor("nodes_T", [6, ROWS], dt, kind="ExternalInput")
    hT_init = nc.dram_tensor("hT_init", [R, NL], dt, kind="ExternalInput")
    c_init = nc.dram_tensor("c_init", [NL, R], dt, kind="ExternalInput")
    w_in_d = nc.dram_tensor("w_in", [6, E], dt, kind="ExternalInput")
    w_t_d = nc.dram_tensor("w_t", [S, E], dt, kind="ExternalInput")
    w_tv_d = nc.dram_tensor("w_tv", [S, E], dt, kind="ExternalInput")
    b_in_d = nc.dram_tensor("b_in_col", [E, 1], dt, kind="ExternalInput")
    b_t_d = nc.dram_tensor("b_t_col", [E, 1], dt, kind="ExternalInput")
    b_tv_d = nc.dram_tensor("b_tv_col", [E, 1], dt, kind="ExternalInput")
    w_ih_d = nc.dram_tensor("w_ih", [3 * E, 4 * R], dt, kind="ExternalInput")
    w_hh_d = nc.dram_tensor("w_hh", [R, 4 * R], dt, kind="ExternalInput")
    b_sum_d = nc.dram_tensor("b_sum", [1, 4 * R], dt, kind="ExternalInput")
    w_out_d = nc.dram_tensor("w_out", [R, O], dt, kind="ExternalInput")
    b_out_d = nc.dram_tensor("b_out_row", [1, O], dt, kind="ExternalInput")
    ident_d = nc.dram_tensor("ident", [128, 128], dt, kind="ExternalInput")
    ones_d = nc.dram_tensor("ones_row", [1, NL], dt, kind="ExternalInput")

    outs_d = nc.dram_tensor("outs", [NL, T * O], dt, kind="ExternalOutput")
    h_d = nc.dram_tensor("h_out", [NL, R], dt, kind="ExternalOutput")
    c_d = nc.dram_tensor("c_out", [NL, R], dt, kind="ExternalOutput")

    with tile.TileContext(nc) as tc:
        import contextlib
        with contextlib.ExitStack() as ctx:
            consts = ctx.enter_context(tc.tile_pool(name="consts", bufs=1))
            grids = ctx.enter_context(tc.tile_pool(name="grids", bufs=3))
            gridsv = ctx.enter_context(tc.tile_pool(name="gridsv", bufs=3))
            tmp = ctx.enter_context(tc.tile_pool(name="tmp", bufs=3))
            lstm = ctx.enter_context(tc.tile_pool(name="lstm", bufs=2))
            ps_gates = ctx.enter_context(
                tc.tile_pool(name="ps_gates", bufs=2, space="PSUM"))
            ps_small = ctx.enter_context(
                tc.tile_pool(name="ps_small", bufs=4, space="PSUM"))

            # ---- load constants ----
            def cload(name, dram, shape):
                t_ = consts.tile(shape, dt, tag=name)
                nc.sync.dma_start(t_[:], dram[:])
                return t_

            w_in = cload("w_in", w_in_d, [6, E])
            w_t = cload("w_t", w_t_d, [S, E])
            w_tv = cload("w_tv", w_tv_d, [S, E])
            b_in = cload("b_in", b_in_d, [E, 1])
            b_t = cload("b_t", b_t_d, [E, 1])
            b_tv = cload("b_tv", b_tv_d, [E, 1])
            w_ih = cload("w_ih", w_ih_d, [3 * E, 4 * R])   # 3 k-slabs stacked: [384, 1024] -> partition=E within slab? NO:
            # w_ih tile is [384,1024] but SBUF partition max 128 -> must split
            # (handled below by allocating three separate tiles instead)
            del w_ih
            w_ih0 = consts.tile([E, 4 * R], dt, tag="w_ih0")
            nc.sync.dma_start(w_ih0[:], w_ih_d[0:E, :])
            w_ih1 = consts.tile([E, 4 * R], dt, tag="w_ih1")
            nc.sync.dma_start(w_ih1[:], w_ih_d[E:2 * E, :])
            w_ih2 = consts.tile([E, 4 * R], dt, tag="w_ih2")
            nc.sync.dma_start(w_ih2[:], w_ih_d[2 * E:3 * E, :])
            w_hh0 = consts.tile([128, 4 * R], dt, tag="w_hh0")
            nc.sync.dma_start(w_hh0[:], w_hh_d[0:128, :])
            w_hh1 = consts.tile([128, 4 * R], dt, tag="w_hh1")
            nc.sync.dma_start(w_hh1[:], w_hh_d[128:256, :])
            b_sum = cload("b_sum", b_sum_d, [1, 4 * R])
            w_out0 = consts.tile([128, O], dt, tag="w_out0")
            nc.sync.dma_start(w_out0[:], w_out_d[0:128, :])
            w_out1 = consts.tile([128, O], dt, tag="w_out1")
            nc.sync.dma_start(w_out1[:], w_out_d[128:256, :])
            b_out = cload("b_out", b_out_d, [1, O])
            ident = cload("ident", ident_d, [128, 128])
            ones = cload("ones", ones_d, [1, NL])
            nodesT = cload("nodesT", nodes_T, [6, ROWS])

            # persistent activations (transposed, rows on free dim)
            eT_in = consts.tile([E, ROWS], dt, tag="eT_in")
            eT_t = consts.tile([E, ROWS], dt, tag="eT_t")
            eT_tv = consts.tile([E, ROWS], dt, tag="eT_tv")
            socT = consts.tile([S, ROWS], dt, tag="socT")
            socvT = consts.tile([S, ROWS], dt, tag="socvT")
            outs_sb = consts.tile([NL, T * O], dt, tag="outs_sb")

            for _rep in range(repeat):
                # ---- e_in for all rows upfront ----
                for n0 in range(0, ROWS, 512):
                    nn = min(512, ROWS - n0)
                    ps = ps_small.tile([128, 512], dt, tag="ps")
                    nc.tensor.matmul(ps[:, :nn], w_in[:], nodesT[:, n0:n0 + nn],
                                     start=True, stop=True)
                    nc.scalar.activation(eT_in[:, n0:n0 + nn], ps[:, :nn],
                                         AF.Relu, bias=b_in[:], scale=1.0)

                # ---- LSTM state init ----
                hT0 = lstm.tile([128, NL], dt, tag="hT0")
                nc.sync.dma_start(hT0[:], hT_init[0:128, :])
                hT1 = lstm.tile([128, NL], dt, tag="hT1")
                nc.sync.dma_start(hT1[:], hT_init[128:256, :])
                c_cur = lstm.tile([NL, R], dt, tag="c")
                nc.sync.dma_start(c_cur[:], c_init[:])

                h_new = None
                for t in range(T):
                    cols = slice(t * NL, (t + 1) * NL)

                    # ---- stream + reduce ped grid ----
                    G = grids.tile([128, PFREE], GRID_DT, tag="G")
                    nc.sync.dma_start(G[:], g_ped[t])
                    part = tmp.tile([128, S], dt, tag="part")
                    nc.vector.tensor_reduce(
                        part[:], G[:].rearrange("p (j s) -> p s j", s=S),
                        axis=AX.X, op=ALU.max)
                    psT = ps_small.tile([S, 128], dt, tag="ps")
                    nc.tensor.transpose(psT[:], part[:], ident[:])
                    nc.vector.tensor_max(socT[:, cols], psT[:, 0:NL],
                                         psT[:, NL:2 * NL])

                    # ---- stream + reduce veh grid ----
                    Gv = gridsv.tile([128, VFREE], GRID_DT, tag="Gv")
                    nc.sync.dma_start(Gv[:], g_veh[t])
                    partv = tmp.tile([128, S], dt, tag="partv")
                    nc.vector.tensor_reduce(
                        partv[:], Gv[:].rearrange("p (j s) -> p s j", s=S),
                        axis=AX.X, op=ALU.max)
                    psTv = ps_small.tile([S, 128], dt, tag="ps")
                    nc.tensor.transpose(psTv[:], partv[:], ident[:])
                    nc.vector.tensor_max(socvT[:, cols], psTv[:, 0:NL],
                                         psTv[:, NL:2 * NL])

                    # ---- social embeddings (transposed) ----
                    pse = ps_small.tile([E, NL], dt, tag="ps")
                    nc.tensor.matmul(pse[:], w_t[:], socT[:, cols],
                                     start=True, stop=True)
                    nc.scalar.activation(eT_t[:, cols], pse[:], AF.Relu,
                                         bias=b_t[:], scale=1.0)
                    psev = ps_small.tile([E, NL], dt, tag="ps")
                    nc.tensor.matmul(psev[:], w_tv[:], socvT[:, cols],
                                     start=True, stop=True)
                    nc.scalar.activation(eT_tv[:, cols], psev[:], AF.Relu,
                                         bias=b_tv[:], scale=1.0)

                    # ---- gates = x @ W_ih + b + h @ W_hh  (4 chunks of 256) ----
                    gates = ps_gates.tile([NL, 4 * R], dt, tag="gates")
                    for gch in range(4):
                        nsl = slice(gch * 256, (gch + 1) * 256)
                        nc.tensor.matmul(gates[:, nsl], eT_in[:, cols],
                                         w_ih0[:, nsl], start=True, stop=False)
                        nc.tensor.matmul(gates[:, nsl], eT_t[:, cols],
                                         w_ih1[:, nsl], start=False, stop=False)
                        nc.tensor.matmul(gates[:, nsl], eT_tv[:, cols],
                                         w_ih2[:, nsl], start=False, stop=False)
                        nc.tensor.matmul(gates[:, nsl], ones[:],
                                         b_sum[:, nsl], start=False, stop=False)
                        nc.tensor.matmul(gates[:, nsl], hT0[:],
                                         w_hh0[:, nsl], start=False, stop=False)
                        nc.tensor.matmul(gates[:, nsl], hT1[:],
                                         w_hh1[:, nsl], start=False, stop=True)

                    # ---- LSTM cell (torch gate order i,f,g,o) ----
                    sig_i = tmp.tile([NL, R], dt, tag="sig_i")
                    nc.scalar.activation(sig_i[:], gates[:, 0:R], AF.Sigmoid)
                    sig_f = tmp.tile([NL, R], dt, tag="sig_f")
                    nc.scalar.activation(sig_f[:], gates[:, R:2 * R], AF.Sigmoid)
                    tan_g = tmp.tile([NL, R], dt, tag="tan_g")
                    nc.scalar.activation(tan_g[:], gates[:, 2 * R:3 * R], AF.Tanh)
                    sig_o = tmp.tile([NL, R], dt, tag="sig_o")
                    nc.scalar.activation(sig_o[:], gates[:, 3 * R:4 * R], AF.Sigmoid)

                    t1_ = tmp.tile([NL, R], dt, tag="t1")
                    nc.vector.tensor_mul(t1_[:], sig_f[:], c_cur[:])
                    t2_ = tmp.tile([NL, R], dt, tag="t2")
                    nc.vector.tensor_mul(t2_[:], sig_i[:], tan_g[:])
                    c_new = lstm.tile([NL, R], dt, tag="c")
                    nc.vector.tensor_add(c_new[:], t1_[:], t2_[:])
                    tan_c = tmp.tile([NL, R], dt, tag="tan_c")
                    nc.scalar.activation(tan_c[:], c_new[:], AF.Tanh)
                    h_new = lstm.tile([NL, R], dt, tag="h")
                    nc.vector.tensor_mul(h_new[:], sig_o[:], tan_c[:])

                    # ---- transpose h for next step / output head ----
                    psh0 = ps_small.tile([128, NL], dt, tag="ps")
                    nc.tensor.transpose(psh0[:], h_new[:, 0:128],
                                        ident[0:NL, 0:NL])
                    hT0 = lstm.tile([128, NL], dt, tag="hT0")
                    nc.scalar.copy(hT0[:], psh0[:])
                    psh1 = ps_small.tile([128, NL], dt, tag="ps")
                    nc.tensor.transpose(psh1[:], h_new[:, 128:256],
                                        ident[0:NL, 0:NL])
                    hT1 = lstm.tile([128, NL], dt, tag="hT1")
                    nc.scalar.copy(hT1[:], psh1[:])

                    # ---- out_t = h @ W_out + b_out ----
                    pso = ps_small.tile([NL, O], dt, tag="ps")
                    nc.tensor.matmul(pso[:], hT0[:], w_out0[:],
                                     start=True, stop=False)
                    nc.tensor.matmul(pso[:], hT1[:], w_out1[:],
                                     start=False, stop=False)
                    nc.tensor.matmul(pso[:], ones[:], b_out[:],
                                     start=False, stop=True)
                    nc.scalar.copy(outs_sb[:, t * O:(t + 1) * O], pso[:])

                    c_cur = c_new

            # ---- writeback ----
            nc.sync.dma_start(outs_d[:], outs_sb[:])
            nc.sync.dma_start(h_d[:], h_new[:])
            nc.sync.dma_start(c_d[:], c_cur[:])

    nc.compile()
    _NC_CACHE[repeat] = nc
    return nc


def shard_inputs(inputs):
    """Full inputs -> list of 8 per-core input maps (numpy, C-contiguous)."""
    f32 = np.float32
    inp = np.asarray(inputs["input_data"], f32)
    gttc = np.asarray(inputs["grids_TTC"], f32)
    gttcv = np.asarray(inputs["grids_TTC_veh"], f32)
    h0 = np.asarray(inputs["hidden_states"], f32)
    c0 = np.asarray(inputs["cell_states"], f32)

    w_in = np.ascontiguousarray(np.asarray(inputs["W_in"], f32))
    w_t = np.ascontiguousarray(np.asarray(inputs["W_t"], f32))
    w_tv = np.ascontiguousarray(np.asarray(inputs["W_tv"], f32))
    w_ih = np.ascontiguousarray(np.asarray(inputs["W_ih"], f32))
    w_hh = np.ascontiguousarray(np.asarray(inputs["W_hh"], f32))
    w_out = np.ascontiguousarray(np.asarray(inputs["W_out"], f32))
    b_in = np.ascontiguousarray(np.asarray(inputs["b_in"], f32).reshape(E, 1))
    b_t = np.ascontiguousarray(np.asarray(inputs["b_t"], f32).reshape(E, 1))
    b_tv = np.ascontiguousarray(np.asarray(inputs["b_tv"], f32).reshape(E, 1))
    b_sum = np.ascontiguousarray(
        (np.asarray(inputs["b_ih"], f32) + np.asarray(inputs["b_hh"], f32))
        .reshape(1, 4 * R))
    b_out = np.ascontiguousarray(np.asarray(inputs["b_out"], f32).reshape(1, O))
    ident = np.eye(128, dtype=f32)
    ones = np.ones((1, NL), f32)

    shared = dict(w_in=w_in, w_t=w_t, w_tv=w_tv, w_ih=w_ih, w_hh=w_hh,
                  w_out=w_out, b_in_col=b_in, b_t_col=b_t, b_tv_col=b_tv,
                  b_sum=b_sum, b_out_row=b_out, ident=ident, ones_row=ones)

    in_maps = []
    for d in range(NCORES):
        i0 = d * NL
        gp = gttc[:, i0:i0 + NL]                       # [T, 64, 512, 24]
        gp = np.ascontiguousarray(
            gp.reshape(T, NL, 2, PFREE).transpose(0, 2, 1, 3)
            .reshape(T, 128, PFREE).astype(GRID_NP))
        gv = gttcv[:, i0:i0 + NL]                      # [T, 64, 64, 24]
        gv = np.ascontiguousarray(
            gv.reshape(T, NL, 2, VFREE).transpose(0, 2, 1, 3)
            .reshape(T, 128, VFREE).astype(GRID_NP))
        nd = inp[:, i0:i0 + NL][:, :, [0, 1, 5, 6, 7, 8]]  # [T, 64, 6]
        nodes_T = np.ascontiguousarray(nd.reshape(ROWS, 6).T)
        in_maps.append(dict(
            g_ped=gp, g_veh=gv, nodes_T=nodes_T,
            hT_init=np.ascontiguousarray(h0[i0:i0 + NL].T),
            c_init=np.ascontiguousarray(c0[i0:i0 + NL]),
            **shared))
    return in_maps


def gather_outputs(results):
    outs, hs, cs = [], [], []
    for r in results:
        outs.append(r["outs"].reshape(NL, T, O).transpose(1, 0, 2))
        hs.append(r["h_out"])
        cs.append(r["c_out"])
    return (np.concatenate(outs, axis=1),
            np.concatenate(hs, axis=0),
            np.concatenate(cs, axis=0))


def kernel(**inputs):
    from concourse.bass_utils import run_bass_kernel_spmd
    nc = build_nc()
    in_maps = shard_inputs(inputs)
    res = run_bass_kernel_spmd(nc, in_maps, core_ids=list(range(NCORES)))
    return gather_outputs(res.results)


# revision 2
# speedup vs baseline: 1.2238x; 1.2238x over previous
"""Trainium2 Bass kernel for a collision-grid social-LSTM model.

Math per frame t (N=512 agents, V=64 vehicles):
  social   = max_j grids_TTC[t, :, j, :]          # [N, 24]
  social_v = max_j grids_TTC_veh[t, :, j, :]      # [N, 24]
  e_in = relu(nodes @ W_in + b_in)                # nodes = input_data[:, [0,1,5..8]]
  e_t  = relu(social @ W_t + b_t)
  e_tv = relu(social_v @ W_tv + b_tv)
  gates = [e_in e_t e_tv] @ W_ih + b_ih + h @ W_hh + b_hh
  LSTM cell (i,f,g,o) -> h, c;  out = h @ W_out + b_out

Sharding: agent dim N split across 8 NeuronCores (64 rows each); weights
replicated; T-scan sequential per core; no collectives needed.

Per-core grid layout: frame slab [64i, 512j, 24s] is reshaped on host to
[128, 6144] with partition p = (j_half*64 + i) so the DMA is contiguous and
the j-reduction uses all 128 partitions.  A PE transpose + elementwise max
merges the two j-halves and yields social^T [24, 64] directly, which is the
operand orientation every downstream matmul wants.
"""

import numpy as np

import concourse.tile as tile
from concourse import bacc, mybir

T, N, V = 19, 512, 64
F, E, R, O = 9, 128, 256, 5
S = 24
NCORES = 8
NL = N // NCORES          # 64 agent rows per core
ROWS = T * NL             # 1216 (t-major row index = t*NL + i)
PFREE = (N // 2) * S      # 6144 free elems per partition (ped)
VFREE = (V // 2) * S      # 768 (veh)

DT = mybir.dt.float32
GRID_DT = mybir.dt.float32   # dtype grids are staged in device DRAM
GRID_NP = np.float32

_NC_CACHE = {}


def build_nc(repeat=1):
    """Build + compile the per-core Bass module (identical on all cores)."""
    if repeat in _NC_CACHE:
        return _NC_CACHE[repeat]

    nc = bacc.Bacc("TRN2", target_bir_lowering=False, debug=False,
                   num_devices=NCORES)
    dt = DT
    AF = mybir.ActivationFunctionType
    ALU = mybir.AluOpType
    AX = mybir.AxisListType

    # ---- DRAM I/O ----
    g_ped = nc.dram_tensor("g_ped", [T, 128, PFREE], GRID_DT, kind="ExternalInput")
    g_veh = nc.dram_tensor("g_veh", [T, 128, VFREE], GRID_DT, kind="ExternalInput")
    nodes_T = nc.dram_tensor("nodes_T", [6, ROWS], dt, kind="ExternalInput")
    hT_init = nc.dram_tensor("hT_init", [R, NL], dt, kind="ExternalInput")
    c_init = nc.dram_tensor("c_init", [NL, R], dt, kind="ExternalInput")
    w_in_d = nc.dram_tensor("w_in", [6, E], dt, kind="ExternalInput")
    w_t_d = nc.dram_tensor("w_t", [S, E], dt, kind="ExternalInput")
    w_tv_d = nc.dram_tensor("w_tv", [S, E], dt, kind="ExternalInput")
    b_in_d = nc.dram_tensor("b_in_col", [E, 1], dt, kind="ExternalInput")
    b_t_d = nc.dram_tensor("b_t_col", [E, 1], dt, kind="ExternalInput")
    b_tv_d = nc.dram_tensor("b_tv_col", [E, 1], dt, kind="ExternalInput")
    w_ih_d = nc.dram_tensor("w_ih", [3 * E, 4 * R], dt, kind="ExternalInput")
    w_hh_d = nc.dram_tensor("w_hh", [R, 4 * R], dt, kind="ExternalInput")
    b_sum_d = nc.dram_tensor("b_sum", [1, 4 * R], dt, kind="ExternalInput")
    w_out_d = nc.dram_tensor("w_out", [R, O], dt, kind="ExternalInput")
    b_out_d = nc.dram_tensor("b_out_row", [1, O], dt, kind="ExternalInput")
    ident_d = nc.dram_tensor("ident", [128, 128], dt, kind="ExternalInput")
    ones_d = nc.dram_tensor("ones_row", [1, NL], dt, kind="ExternalInput")

    outs_d = nc.dram_tensor("outs", [NL, T * O], dt, kind="ExternalOutput")
    h_d = nc.dram_tensor("h_out", [NL, R], dt, kind="ExternalOutput")
    c_d = nc.dram_tensor("c_out", [NL, R], dt, kind="ExternalOutput")

    with tile.TileContext(nc) as tc:
        import contextlib
        with contextlib.ExitStack() as ctx:
            consts = ctx.enter_context(tc.tile_pool(name="consts", bufs=1))
            grids = ctx.enter_context(tc.tile_pool(name="grids", bufs=3))
            gridsv = ctx.enter_context(tc.tile_pool(name="gridsv", bufs=3))
            tmp = ctx.enter_context(tc.tile_pool(name="tmp", bufs=3))
            lstm = ctx.enter_context(tc.tile_pool(name="lstm", bufs=2))
            ps_gates = ctx.enter_context(
                tc.tile_pool(name="ps_gates", bufs=2, space="PSUM"))
            ps_small = ctx.enter_context(
                tc.tile_pool(name="ps_small", bufs=4, space="PSUM"))

            # ---- load constants ----
            def cload(name, dram, shape):
                t_ = consts.tile(shape, dt, tag=name)
                nc.sync.dma_start(t_[:], dram[:])
                return t_

            w_in = cload("w_in", w_in_d, [6, E])
            w_t = cload("w_t", w_t_d, [S, E])
            w_tv = cload("w_tv", w_tv_d, [S, E])
            b_in = cload("b_in", b_in_d, [E, 1])
            b_t = cload("b_t", b_t_d, [E, 1])
            b_tv = cload("b_tv", b_tv_d, [E, 1])
            # W_ih is [384, 1024]: split into three [128, 1024] k-slabs
            w_ih0 = consts.tile([E, 4 * R], dt, tag="w_ih0")
            nc.sync.dma_start(w_ih0[:], w_ih_d[0:E, :])
            w_ih1 = consts.tile([E, 4 * R], dt, tag="w_ih1")
            nc.sync.dma_start(w_ih1[:], w_ih_d[E:2 * E, :])
            w_ih2 = consts.tile([E, 4 * R], dt, tag="w_ih2")
            nc.sync.dma_start(w_ih2[:], w_ih_d[2 * E:3 * E, :])
            w_hh0 = consts.tile([128, 4 * R], dt, tag="w_hh0")
            nc.sync.dma_start(w_hh0[:], w_hh_d[0:128, :])
            w_hh1 = consts.tile([128, 4 * R], dt, tag="w_hh1")
            nc.sync.dma_start(w_hh1[:], w_hh_d[128:256, :])
            b_sum = cload("b_sum", b_sum_d, [1, 4 * R])
            w_out0 = consts.tile([128, O], dt, tag="w_out0")
            nc.sync.dma_start(w_out0[:], w_out_d[0:128, :])
            w_out1 = consts.tile([128, O], dt, tag="w_out1")
            nc.sync.dma_start(w_out1[:], w_out_d[128:256, :])
            b_out = cload("b_out", b_out_d, [1, O])
            ident = cload("ident", ident_d, [128, 128])
            ones = cload("ones", ones_d, [1, NL])
            nodesT = cload("nodesT", nodes_T, [6, ROWS])

            # persistent activations (transposed, rows on free dim)
            eT_in = consts.tile([E, ROWS], dt, tag="eT_in")
            eT_t = consts.tile([E, ROWS], dt, tag="eT_t")
            eT_tv = consts.tile([E, ROWS], dt, tag="eT_tv")
            socT = consts.tile([S, ROWS], dt, tag="socT")
            socvT = consts.tile([S, ROWS], dt, tag="socvT")
            outs_sb = consts.tile([NL, T * O], dt, tag="outs_sb")

            for _rep in range(repeat):
                # ---- e_in for all rows upfront ----
                for n0 in range(0, ROWS, 512):
                    nn = min(512, ROWS - n0)
                    ps = ps_small.tile([128, 512], dt, tag="ps")
                    nc.tensor.matmul(ps[:, :nn], w_in[:], nodesT[:, n0:n0 + nn],
                                     start=True, stop=True)
                    nc.scalar.activation(eT_in[:, n0:n0 + nn], ps[:, :nn],
                                         AF.Relu, bias=b_in[:], scale=1.0)

                # ---- LSTM state init ----
                hT0 = lstm.tile([128, NL], dt, tag="hT0")
                nc.sync.dma_start(hT0[:], hT_init[0:128, :])
                hT1 = lstm.tile([128, NL], dt, tag="hT1")
                nc.sync.dma_start(hT1[:], hT_init[128:256, :])
                c_cur = lstm.tile([NL, R], dt, tag="c")
                nc.sync.dma_start(c_cur[:], c_init[:])

                h_new = None
                for t in range(T):
                    cols = slice(t * NL, (t + 1) * NL)

                    # ---- stream + reduce ped grid ----
                    G = grids.tile([128, PFREE], GRID_DT, tag="G")
                    nc.sync.dma_start(G[:], g_ped[t])
                    part = tmp.tile([128, S], dt, tag="part")
                    nc.vector.tensor_reduce(
                        part[:], G[:].rearrange("p (j s) -> p s j", s=S),
                        axis=AX.X, op=ALU.max)
                    psT = ps_small.tile([S, 128], dt, tag="ps")
                    nc.tensor.transpose(psT[:], part[:], ident[:])
                    nc.vector.tensor_max(socT[:, cols], psT[:, 0:NL],
                                         psT[:, NL:2 * NL])

                    # ---- stream + reduce veh grid ----
                    Gv = gridsv.tile([128, VFREE], GRID_DT, tag="Gv")
                    nc.sync.dma_start(Gv[:], g_veh[t])
                    partv = tmp.tile([128, S], dt, tag="partv")
                    nc.vector.tensor_reduce(
                        partv[:], Gv[:].rearrange("p (j s) -> p s j", s=S),
                        axis=AX.X, op=ALU.max)
                    psTv = ps_small.tile([S, 128], dt, tag="ps")
                    nc.tensor.transpose(psTv[:], partv[:], ident[:])
                    nc.vector.tensor_max(socvT[:, cols], psTv[:, 0:NL],
                                         psTv[:, NL:2 * NL])

                    # ---- social embeddings (transposed) ----
                    pse = ps_small.tile([E, NL], dt, tag="ps")
                    nc.tensor.matmul(pse[:], w_t[:], socT[:, cols],
                                     start=True, stop=True)
                    nc.scalar.activation(eT_t[:, cols], pse[:], AF.Relu,
                                         bias=b_t[:], scale=1.0)
                    psev = ps_small.tile([E, NL], dt, tag="ps")
                    nc.tensor.matmul(psev[:], w_tv[:], socvT[:, cols],
                                     start=True, stop=True)
                    nc.scalar.activation(eT_tv[:, cols], psev[:], AF.Relu,
                                         bias=b_tv[:], scale=1.0)

                    # ---- gates = x @ W_ih + b + h @ W_hh  (4 chunks of 256) ----
                    gates = ps_gates.tile([NL, 4 * R], dt, tag="gates")
                    for gch in range(4):
                        nsl = slice(gch * 256, (gch + 1) * 256)
                        nc.tensor.matmul(gates[:, nsl], eT_in[:, cols],
                                         w_ih0[:, nsl], start=True, stop=False)
                        nc.tensor.matmul(gates[:, nsl], eT_t[:, cols],
                                         w_ih1[:, nsl], start=False, stop=False)
                        nc.tensor.matmul(gates[:, nsl], eT_tv[:, cols],
                                         w_ih2[:, nsl], start=False, stop=False)
                        nc.tensor.matmul(gates[:, nsl], ones[:],
                                         b_sum[:, nsl], start=False, stop=False)
                        nc.tensor.matmul(gates[:, nsl], hT0[:],
                                         w_hh0[:, nsl], start=False, stop=False)
                        nc.tensor.matmul(gates[:, nsl], hT1[:],
                                         w_hh1[:, nsl], start=False, stop=True)

                    # ---- LSTM cell (torch gate order i,f,g,o) ----
                    sig_i = tmp.tile([NL, R], dt, tag="sig_i")
                    nc.scalar.activation(sig_i[:], gates[:, 0:R], AF.Sigmoid)
                    sig_f = tmp.tile([NL, R], dt, tag="sig_f")
                    nc.scalar.activation(sig_f[:], gates[:, R:2 * R], AF.Sigmoid)
                    tan_g = tmp.tile([NL, R], dt, tag="tan_g")
                    nc.scalar.activation(tan_g[:], gates[:, 2 * R:3 * R], AF.Tanh)
                    sig_o = tmp.tile([NL, R], dt, tag="sig_o")
                    nc.scalar.activation(sig_o[:], gates[:, 3 * R:4 * R], AF.Sigmoid)

                    t1_ = tmp.tile([NL, R], dt, tag="t1")
                    nc.vector.tensor_mul(t1_[:], sig_f[:], c_cur[:])
                    t2_ = tmp.tile([NL, R], dt, tag="t2")
                    nc.vector.tensor_mul(t2_[:], sig_i[:], tan_g[:])
                    c_new = lstm.tile([NL, R], dt, tag="c")
                    nc.vector.tensor_add(c_new[:], t1_[:], t2_[:])
                    tan_c = tmp.tile([NL, R], dt, tag="tan_c")
                    nc.scalar.activation(tan_c[:], c_new[:], AF.Tanh)
                    h_new = lstm.tile([NL, R], dt, tag="h")
                    nc.vector.tensor_mul(h_new[:], sig_o[:], tan_c[:])

                    # ---- transpose h for next step / output head ----
                    psh0 = ps_small.tile([128, NL], dt, tag="ps")
                    nc.tensor.transpose(psh0[:], h_new[:, 0:128],
                                        ident[0:NL, 0:NL])
                    hT0 = lstm.tile([128, NL], dt, tag="hT0")
                    nc.scalar.copy(hT0[:], psh0[:])
                    psh1 = ps_small.tile([128, NL], dt, tag="ps")
                    nc.tensor.transpose(psh1[:], h_new[:, 128:256],
                                        ident[0:NL, 0:NL])
                    hT1 = lstm.tile([128, NL], dt, tag="hT1")
                    nc.scalar.copy(hT1[:], psh1[:])

                    # ---- out_t = h @ W_out + b_out ----
                    pso = ps_small.tile([NL, O], dt, tag="ps")
                    nc.tensor.matmul(pso[:], hT0[:], w_out0[:],
                                     start=True, stop=False)
                    nc.tensor.matmul(pso[:], hT1[:], w_out1[:],
                                     start=False, stop=False)
                    nc.tensor.matmul(pso[:], ones[:], b_out[:],
                                     start=False, stop=True)
                    nc.scalar.copy(outs_sb[:, t * O:(t + 1) * O], pso[:])

                    c_cur = c_new

            # ---- writeback ----
            nc.sync.dma_start(outs_d[:], outs_sb[:])
            nc.sync.dma_start(h_d[:], h_new[:])
            nc.sync.dma_start(c_d[:], c_cur[:])

    nc.compile()
    _NC_CACHE[repeat] = nc
    return nc


def shard_inputs(inputs):
    """Full inputs -> list of 8 per-core input maps (numpy, C-contiguous)."""
    f32 = np.float32
    inp = np.asarray(inputs["input_data"], f32)
    gttc = np.asarray(inputs["grids_TTC"], f32)
    gttcv = np.asarray(inputs["grids_TTC_veh"], f32)
    h0 = np.asarray(inputs["hidden_states"], f32)
    c0 = np.asarray(inputs["cell_states"], f32)

    w_in = np.ascontiguousarray(np.asarray(inputs["W_in"], f32))
    w_t = np.ascontiguousarray(np.asarray(inputs["W_t"], f32))
    w_tv = np.ascontiguousarray(np.asarray(inputs["W_tv"], f32))
    w_ih = np.ascontiguousarray(np.asarray(inputs["W_ih"], f32))
    w_hh = np.ascontiguousarray(np.asarray(inputs["W_hh"], f32))
    w_out = np.ascontiguousarray(np.asarray(inputs["W_out"], f32))
    b_in = np.ascontiguousarray(np.asarray(inputs["b_in"], f32).reshape(E, 1))
    b_t = np.ascontiguousarray(np.asarray(inputs["b_t"], f32).reshape(E, 1))
    b_tv = np.ascontiguousarray(np.asarray(inputs["b_tv"], f32).reshape(E, 1))
    b_sum = np.ascontiguousarray(
        (np.asarray(inputs["b_ih"], f32) + np.asarray(inputs["b_hh"], f32))
        .reshape(1, 4 * R))
    b_out = np.ascontiguousarray(np.asarray(inputs["b_out"], f32).reshape(1, O))
    ident = np.eye(128, dtype=f32)
    ones = np.ones((1, NL), f32)

    shared = dict(w_in=w_in, w_t=w_t, w_tv=w_tv, w_ih=w_ih, w_hh=w_hh,
                  w_out=w_out, b_in_col=b_in, b_t_col=b_t, b_tv_col=b_tv,
                  b_sum=b_sum, b_out_row=b_out, ident=ident, ones_row=ones)

    in_maps = []
    for d in range(NCORES):
        i0 = d * NL
        gp = gttc[:, i0:i0 + NL]                       # [T, 64, 512, 24]
        gp = np.ascontiguousarray(
            gp.reshape(T, NL, 2, PFREE).transpose(0, 2, 1, 3)
            .reshape(T, 128, PFREE).astype(GRID_NP))
        gv = gttcv[:, i0:i0 + NL]                      # [T, 64, 64, 24]
        gv = np.ascontiguousarray(
            gv.reshape(T, NL, 2, VFREE).transpose(0, 2, 1, 3)
            .reshape(T, 128, VFREE).astype(GRID_NP))
        nd = inp[:, i0:i0 + NL][:, :, [0, 1, 5, 6, 7, 8]]  # [T, 64, 6]
        nodes_T = np.ascontiguousarray(nd.reshape(ROWS, 6).T)
        in_maps.append(dict(
            g_ped=gp, g_veh=gv, nodes_T=nodes_T,
            hT_init=np.ascontiguousarray(h0[i0:i0 + NL].T),
            c_init=np.ascontiguousarray(c0[i0:i0 + NL]),
            **shared))
    return in_maps


def gather_outputs(results):
    outs, hs, cs = [], [], []
    for r in results:
        outs.append(r["outs"].reshape(NL, T, O).transpose(1, 0, 2))
        hs.append(r["h_out"])
        cs.append(r["c_out"])
    return (np.concatenate(outs, axis=1),
            np.concatenate(hs, axis=0),
            np.concatenate(cs, axis=0))


def kernel(**inputs):
    from concourse.bass_utils import run_bass_kernel_spmd
    nc = build_nc()
    in_maps = shard_inputs(inputs)
    res = run_bass_kernel_spmd(nc, in_maps, core_ids=list(range(NCORES)))
    return gather_outputs(res.results)
